# revision 1
# baseline (speedup 1.0000x reference)
"""Trainium2 Bass kernel for the MoE-routing module.

Computation (B=32768, D=1024, H=512, F=100, E=16, K=2):
    h   = relu(x @ W_shared + b_shared)                  [B, H]
    a   = relu(einsum('bh,ehf', h, W1) + b1)             [B, E, F]
    o   = einsum('bef,efo', a, W2) + b2                  [B, E, 1]
    out = mean over the K routed experts of o[b, send_to[idx[b]]]

Strategy: host sorts tokens by head id and shards the sorted batch over the
8 cores (4096 tokens each, perfectly balanced).  A sorted 4096-token window
only routes to a handful of consecutive experts, so each core gets just the
expert slices it needs (EC slots, adaptively >= actual need; EC=16 degrades
to the dense all-expert kernel).  Routing is folded into a host-computed
per-slot mask M[j, b], so the device computes
    out[b] = sum_j o_local[b, j] * M[j, b]
with three matmul stages, features on SBUF partitions throughout:
  M1: hT[h, t]  = relu(W_shared.T @ xT)         lhsT = W_shared tiles
  M2: aT[f', t] = relu(W1sel.T @ hT)            f' = j*F + f  (EC*F wide)
  M3: c[j, t]   = W2sel.T @ aT                  W2sel block-diagonal
  sel: out[t]   = ones.T @ (c * mask)           1-partition result row
All matmuls run as float32r (full-rate fp32 mode, ~1e-4 rel err).
"""

import os

import numpy as np

import concourse.mybir as mybir
from concourse import bacc
from concourse.bass_utils import run_bass_kernel_spmd
from concourse.tile import TileContext

B, D, H, F, E, TOPK = 32768, 1024, 512, 100, 16, 2
N_CORES = 8
BL = B // N_CORES          # tokens per core
CHUNK = 512                # tokens per device-side tile loop
N_CHUNKS = BL // CHUNK
MH = H // 128              # M1 output tiles
KD = D // 128              # M1 contraction tiles
KH = H // 128              # M2 contraction tiles
EC_MIN = 5                 # minimum expert slots per core
CHUNK_SIZES = [512] * 8

# Compute dtype for the matmul stages: "float32", "float32r", or "bfloat16"
COMPUTE_DT = os.environ.get("KERNEL_DT", "float32r")

_FP32 = mybir.dt.float32
_cache = {}


def _np_in_dtype():
    import ml_dtypes

    return ml_dtypes.bfloat16 if COMPUTE_DT == "bfloat16" else np.float32


def _build_nc(ec):
    """Build the SPMD program for EC expert slots per core."""
    CDT = getattr(mybir.dt, COMPUTE_DT)
    SDT = mybir.dt.bfloat16 if COMPUTE_DT == "bfloat16" else mybir.dt.float32
    EF = ec * F                    # local expert-concat width
    KT3 = (EF + 127) // 128        # M2 output tiles / M3 contraction tiles
    EF_PAD = KT3 * 128             # w1sel zero-padded so all tiles are full
    NB = MH + KT3 + 1              # packed bias columns

    nc = bacc.Bacc("TRN2", target_bir_lowering=False, num_devices=N_CORES)

    xT_d = nc.declare_dram_parameter("xT", [D * BL], CDT, isOutput=False)
    mask_d = nc.declare_dram_parameter("mask", [33, BL], _FP32, isOutput=False)
    wsh_d = nc.declare_dram_parameter("wsh", [D, H], CDT, isOutput=False)
    w1c_d = nc.declare_dram_parameter("w1c", [H, EF_PAD], CDT, isOutput=False)
    w2bd_d = nc.declare_dram_parameter("w2bd", [128, KT3 * ec], CDT, isOutput=False)
    bias_d = nc.declare_dram_parameter("biases", [128, NB], _FP32, isOutput=False)
    out_d = nc.declare_dram_parameter("out", [BL], _FP32, isOutput=True)

    relu = mybir.ActivationFunctionType.Relu
    sizes = CHUNK_SIZES
    offs = np.cumsum([0] + sizes).tolist()

    with TileContext(nc) as tc:
        with (
            tc.tile_pool(name="weights", bufs=1) as wpool,
            tc.tile_pool(name="xin", bufs=3) as xpool,
            tc.tile_pool(name="mid", bufs=3) as midpool,
            tc.tile_pool(name="small", bufs=3) as spool,
            tc.tile_pool(name="ps_h", bufs=4, space="PSUM") as ps_h,
            tc.tile_pool(name="ps_a", bufs=2, space="PSUM") as ps_a,
            tc.tile_pool(name="ps_c", bufs=1, space="PSUM") as ps_c,
            tc.tile_pool(name="ps_o", bufs=1, space="PSUM") as ps_o,
        ):
            # ---- input DMAs: explicit priorities pin queue order to
            # program order.  Separate tiles per k-piece — Tile dependency
            # tracking is per-tile, so split DMAs into one tile would
            # serialize as write-after-write.  wsh + chunk-0 x interleave
            # across both HWDGE queues so M1 starts after the first ~512KB.
            _prio = [0]

            def pdma(q, dst, src):
                inst = q.dma_start(dst, src)
                inst.ins.bass_priority = _prio[0]
                _prio[0] += 1
                return inst

            def xview(c):
                sz = sizes[c]
                o = offs[c] * D
                return xT_d[o : o + D * sz].rearrange("(ko p t) -> p ko t", p=128, t=sz)

            wsh_view = wsh_d.rearrange("(o p) h -> p o h", p=128)
            wsh_ks = [wpool.tile([128, H], CDT, name=f"wshk{k}") for k in range(KD)]
            xt0_view = xview(0)
            xt0 = [
                xpool.tile([128, CHUNK], CDT, tag=f"xt{k}", name=f"xt0_{k}")
                for k in range(KD)
            ]
            for k in range(KD):
                qa = nc.sync if k % 2 == 0 else nc.scalar
                qb = nc.scalar if k % 2 == 0 else nc.sync
                pdma(qa, wsh_ks[k][:], wsh_view[:, k])
                pdma(qb, xt0[k][:, : sizes[0]], xt0_view[:, k])

            xts, masks = [[t[:, : sizes[0]] for t in xt0]], []
            w1c_ks = [None] * KH
            for c in range(len(sizes)):
                sz = sizes[c]
                if c > 0:
                    xv = xview(c)
                    xa = xpool.tile([128, KD // 2, CHUNK], CDT, tag="xta", name=f"xta{c}")
                    xb = xpool.tile([128, KD // 2, CHUNK], CDT, tag="xtb", name=f"xtb{c}")
                    pdma(nc.scalar, xa[:, :, :sz], xv[:, : KD // 2])
                    pdma(nc.sync, xb[:, :, :sz], xv[:, KD // 2 :])
                    xts.append([xa[:, k, :sz] for k in range(KD // 2)] + [xb[:, k, :sz] for k in range(KD // 2)])
                mask_sb = spool.tile([33, CHUNK], _FP32, tag="mask")
                pdma(nc.scalar, mask_sb[:, :sz], mask_d[:, offs[c] : offs[c] + sz])
                masks.append(mask_sb[:, :sz])
                if c == 0:
                    w1c_view = w1c_d.rearrange("(o p) f -> p o f", p=128)
                    for k in range(KH):
                        w1c_ks[k] = wpool.tile([128, EF_PAD], CDT, name=f"w1ck{k}")
                        pdma(nc.sync if k % 2 == 0 else nc.scalar, w1c_ks[k][:], w1c_view[:, k])
                    w2bd_sb = wpool.tile([128, KT3 * ec], CDT)
                    pdma(nc.sync, w2bd_sb[:], w2bd_d[:])
                    bias_sb = wpool.tile([128, NB], _FP32)
                    pdma(nc.sync, bias_sb[:], bias_d[:])
                    ones_sb = wpool.tile([ec, 1], CDT)
                    if COMPUTE_DT == "float32r":
                        nc.vector.memset(ones_sb[:].bitcast(mybir.dt.float32), 1.0)
                    else:
                        nc.vector.memset(ones_sb[:], 1.0)

            for c in range(len(sizes)):
                sz = sizes[c]
                t0 = offs[c]
                xt = xts[c]
                mask_sb = masks[c]

                # ---- M1: hT = relu(W_shared.T @ xT + b) ----
                # chunk 0 runs k-outer so matmuls start as soon as the first
                # split DMA pieces land; later chunks are fully prefetched.
                hT = midpool.tile([128, MH, CHUNK], CDT, tag="hT", name=f"hT{c}")[:, :, :sz]
                if c == 0:
                    phs = [ps_h.tile([128, CHUNK], _FP32, tag="ps_h", name=f"ph{m}")[:, :sz] for m in range(MH)]
                    for k in range(KD):
                        for m in range(MH):
                            nc.tensor.matmul(
                                phs[m][:],
                                lhsT=wsh_ks[k][:, m * 128 : (m + 1) * 128],
                                rhs=xt[k][:],
                                start=(k == 0),
                                stop=(k == KD - 1),
                            )
                    for m in range(MH):
                        nc.scalar.activation(
                            hT[:, m, :], phs[m][:], relu, bias=bias_sb[:, m : m + 1]
                        )
                else:
                    for m in range(MH):
                        ph = ps_h.tile([128, CHUNK], _FP32, tag="ps_h", name=f"phx{c}_{m}")[:, :sz]
                        for k in range(KD):
                            nc.tensor.matmul(
                                ph[:],
                                lhsT=wsh_ks[k][:, m * 128 : (m + 1) * 128],
                                rhs=xt[k][:],
                                start=(k == 0),
                                stop=(k == KD - 1),
                            )
                        nc.scalar.activation(
                            hT[:, m, :], ph[:], relu, bias=bias_sb[:, m : m + 1]
                        )

                # ---- M2: aT = relu(W1sel.T @ hT + b1) ----
                aT = midpool.tile([128, KT3, CHUNK], CDT, tag="aT", name=f"aT{c}")[:, :, :sz]
                for m in range(KT3):
                    f0 = m * 128
                    pa = ps_a.tile([128, CHUNK], _FP32, tag="ps_a", name=f"pa{c}_{m}")[:, :sz]
                    for k in range(KH):
                        nc.tensor.matmul(
                            pa[:],
                            lhsT=w1c_ks[k][:, f0 : f0 + 128],
                            rhs=hT[:, k, :],
                            start=(k == 0),
                            stop=(k == KH - 1),
                        )
                    nc.scalar.activation(
                        aT[:, m, :], pa[:], relu,
                        bias=bias_sb[:, MH + m : MH + m + 1],
                    )

                # ---- M3: c = W2sel.T @ aT  (block-diag W2) ----
                pc = ps_c.tile([ec, CHUNK], _FP32, tag="ps_c", name=f"pc{c}")[:, :sz]
                for k in range(KT3):
                    nc.tensor.matmul(
                        pc[:],
                        lhsT=w2bd_sb[:, k * ec : (k + 1) * ec],
                        rhs=aT[:, k, :],
                        start=(k == 0),
                        stop=(k == KT3 - 1),
                    )

                # ---- select: out = ones.T @ (c * mask) + btok ----
                msel = spool.tile([ec, CHUNK], CDT, tag="msel", name=f"msel{c}")[:, :sz]
                nc.vector.tensor_mul(msel[:], pc[:], mask_sb[:ec])
                po = ps_o.tile([1, CHUNK], _FP32, tag="ps_o", name=f"po{c}")[:, :sz]
                nc.tensor.matmul(po[:], lhsT=ones_sb[:], rhs=msel[:], start=True, stop=True)
                ot = spool.tile([1, CHUNK], _FP32, tag="ot", name=f"ot{c}")[:, :sz]
                nc.vector.tensor_add(ot[:], po[:], mask_sb[32:33])
                nc.gpsimd.dma_start(out_d[t0 : t0 + sz].rearrange("(o t) -> o t", o=1), ot[:])

    nc.compile()
    return nc


def get_nc(ec):
    key = (COMPUTE_DT, ec)
    if key not in _cache:
        _cache[key] = _build_nc(ec)
    return _cache[key]


def prepare(inputs):
    """Host-side routing/sorting/sharding. Returns (ec, in_maps, perm)."""
    np_dt = _np_in_dtype()
    x = np.asarray(inputs["x"], dtype=np.float32)
    idx = np.asarray(inputs["idx"]).astype(np.int64).reshape(B)
    W_shared = np.asarray(inputs["W_shared"], dtype=np.float32)
    b_shared = np.asarray(inputs["b_shared"], dtype=np.float32).reshape(H)
    W1 = np.asarray(inputs["W1"], dtype=np.float32)
    b1 = np.asarray(inputs["b1"], dtype=np.float32).reshape(E, F)
    W2 = np.asarray(inputs["W2"], dtype=np.float32).reshape(E, F)
    b2 = np.asarray(inputs["b2"], dtype=np.float32).reshape(E)
    send_to = np.asarray(inputs["send_to"]).astype(np.int64)

    perm = np.argsort(idx, kind="stable")
    idx_s = idx[perm]
    routes_s = send_to[idx_s]                      # [B, K] sorted routes
    x_s = x[perm]                                  # [B, D]

    # per-core expert lists
    expert_lists = []
    for c in range(N_CORES):
        sl = slice(c * BL, (c + 1) * BL)
        expert_lists.append(np.unique(routes_s[sl]))
    ec = max(EC_MIN, max(len(el) for el in expert_lists))
    ec = min(ec, E)

    wsh = np.ascontiguousarray(W_shared).astype(np_dt)
    EF = ec * F
    KT3 = (EF + 127) // 128
    EF_PAD = KT3 * 128
    NB = MH + KT3 + 1

    in_maps = []
    for c in range(N_CORES):
        sl = slice(c * BL, (c + 1) * BL)
        el = expert_lists[c]
        # local slot tables (pad slots use sentinel -1: zero weights, no mask)
        slots = np.full(ec, -1, dtype=np.int64)
        slots[: len(el)] = el

        # mask[j, b] = (1/K) * count of slots[j] among routes of token b
        r = routes_s[sl]                            # [BL, K]
        mask = np.zeros((33, BL), dtype=np.float32)
        for k in range(r.shape[1]):
            hit = slots[:, None] == r[None, :, k]   # [ec, BL]
            mask[:ec] += hit.astype(np.float32) / r.shape[1]
        mask[32] = b2[r].mean(axis=1)               # routed-b2 mean per token

        w1sel = np.zeros((H, EF_PAD), dtype=np.float32)
        b1sel = np.zeros(EF_PAD, dtype=np.float32)
        w2full = np.zeros((EF_PAD, ec), dtype=np.float32)
        for j, e in enumerate(slots):
            if e < 0:
                continue
            w1sel[:, j * F : (j + 1) * F] = W1[e]
            b1sel[j * F : (j + 1) * F] = b1[e]
            w2full[j * F : (j + 1) * F, j] = W2[e]
        w2bd = np.ascontiguousarray(
            w2full.reshape(KT3, 128, ec).transpose(1, 0, 2).reshape(128, KT3 * ec)
        ).astype(np_dt)

        biases = np.zeros((128, NB), dtype=np.float32)
        biases[:, :MH] = b_shared.reshape(MH, 128).T
        biases[:, MH : MH + KT3] = b1sel.reshape(KT3, 128).T
        biases[:ec, MH + KT3] = b2[np.maximum(slots, 0)] * (slots >= 0)

        xc = x_s[sl]
        parts, o = [], 0
        for szc in CHUNK_SIZES:
            parts.append(xc[o : o + szc].T.ravel())
            o += szc
        xT = np.ascontiguousarray(np.concatenate(parts)).astype(np_dt)

        in_maps.append(
            {
                "xT": xT,
                "mask": mask,
                "wsh": wsh,
                "w1c": w1sel.astype(np_dt),
                "w2bd": w2bd,
                "biases": biases,
            }
        )
    return ec, in_maps, perm


def kernel(**inputs) -> np.ndarray:
    ec, in_maps, perm = prepare(inputs)
    nc = get_nc(ec)
    res = run_bass_kernel_spmd(nc, in_maps, list(range(N_CORES)))
    out_sorted = np.concatenate([res.results[c]["out"] for c in range(N_CORES)])
    out = np.empty(B, dtype=np.float32)
    out[perm] = out_sorted
    return out.reshape(B, 1)



# revision 5
# speedup vs baseline: 1.2180x; 1.2180x over previous
"""Trainium2 Bass kernel for the MoE-routing module (v2).

Computation (B=32768, D=1024, H=512, F=100, E=16, K=2):
    h   = relu(x @ W_shared + b_shared)                  [B, H]
    a   = relu(einsum('bh,ehf', h, W1) + b1)             [B, E, F]
    o   = einsum('bef,efo', a, W2) + b2                  [B, E, 1]
    out = mean over the K routed experts of o[b, send_to[idx[b]]]

Strategy (v2):
  * Host sorts tokens by head id, cuts the sorted batch into 64 chunks of
    512 tokens.  A chunk touches 2 experts (3 when it straddles a head-id
    boundary; there are <=15 such chunks).  Chunks are dealt to the 8
    cores so that every core sees the same per-position expert-slot
    pattern (typically [3,3,2,2,2,2,2,2]) -> a single SPMD program.
  * Per-core compute, features on SBUF partitions throughout (bf16):
      M1: hT[h, t]  = relu(W_shared.T @ xT)   1024-token superchunks
      M2: aT[f',t]  = relu(W1sel.T @ hT)      f' = chunk-local slot*F + f
      mask fold:  am = aT * mexp   (mexp = routing mask expanded to f')
      M3: out[t]    = sum_k w2col_k.T @ am_k  (single-partition result)
    Folding the mask into the activations removes the separate
    select-matmul and its PSUM bank, freeing banks for wide M1 PSUM.
  * bf16 operands: half the HBM traffic of fp32, same PE column rate,
    and FWL (fast weight load) applies (it is disabled for fp32),
    hiding LDWEIGHTS behind matmuls.
  * Queue discipline: Scalar queue = activations (+ a few early DMA
    issues), Sync queue = bulk DMA issue, GpSimd = output DMA.  v1
    stalled 6.5us with RELUs head-blocked behind DMA descriptors.
  * ~32 warm-up matmuls run during the DMA lead-in so the PE HAM clock
    gate is already at 2.4 GHz when real work arrives.
"""

import os

import numpy as np

import concourse.mybir as mybir
from concourse import bacc
from concourse.bass_utils import run_bass_kernel_spmd
from concourse.tile import TileContext

B, D, H, F, E, TOPK = 32768, 1024, 512, 100, 16, 2
N_CORES = 8
BL = B // N_CORES          # tokens per core (4096)
PS = 512                   # tokens per routing chunk / M2 position
NPOS = BL // PS            # positions per core (8)
MH = H // 128              # M1 output tiles (4)
KD = D // 128              # M1 contraction tiles (8)
KH = H // 128              # M2 contraction tiles (4)
NWARM = int(os.environ.get("KERNEL_WARMUP", "32"))

# Compute dtype for the matmul stages: "float32", "float32r", or "bfloat16"
COMPUTE_DT = os.environ.get("KERNEL_DT", "bfloat16")

_FP32 = mybir.dt.float32
_cache = {}


def _np_in_dtype():
    import ml_dtypes

    return ml_dtypes.bfloat16 if COMPUTE_DT == "bfloat16" else np.float32


def _tch():
    # Matmul output is capped at one PSUM bank (512 fp32 columns), so the
    # token-chunk width is 512 for every compute dtype.
    return 512


def _geom(pattern):
    """Derived geometry for a per-position expert-slot pattern."""
    kt3 = [(ec * F + 127) // 128 for ec in pattern]
    col0, c = [], 0
    for ec in pattern:
        col0.append(c)
        c += ec * F
    w1w = max(col0[p] + kt3[p] * 128 for p in range(len(pattern)))
    boff, b = [], 0
    for k in kt3:
        boff.append(b)
        b += k
    skt = b
    return kt3, col0, w1w, boff, skt


def _build_nc(pattern):
    """Build the SPMD program for the given per-position slot pattern."""
    CDT = getattr(mybir.dt, COMPUTE_DT)
    TCH = _tch()
    NSUP = BL // TCH
    QP = TCH // PS
    kt3, col0, W1W, boff, SKT = _geom(pattern)
    NB = MH + SKT

    nc = bacc.Bacc("TRN2", target_bir_lowering=False, num_devices=N_CORES)

    xT_d = nc.declare_dram_parameter("xT", [D * BL], CDT, isOutput=False)
    wsh_d = nc.declare_dram_parameter("wsh", [D, H], CDT, isOutput=False)
    w1_d = nc.declare_dram_parameter("w1all", [H, W1W], CDT, isOutput=False)
    mexp_d = nc.declare_dram_parameter("mexp", [128, SKT * PS], CDT, isOutput=False)
    w2_d = nc.declare_dram_parameter("w2bd", [128, SKT], CDT, isOutput=False)
    bias_d = nc.declare_dram_parameter("biases", [128, NB], _FP32, isOutput=False)
    b2r_d = nc.declare_dram_parameter("b2row", [1, BL], _FP32, isOutput=False)
    out_d = nc.declare_dram_parameter("out", [BL], _FP32, isOutput=True)

    relu = mybir.ActivationFunctionType.Relu

    with TileContext(nc) as tc:
        with (
            tc.tile_pool(name="weights", bufs=1) as wpool,
            tc.tile_pool(name="xin", bufs=1) as xpool,
            tc.tile_pool(name="mid", bufs=3) as midpool,
            tc.tile_pool(name="act", bufs=2) as apool,
            tc.tile_pool(name="small", bufs=3) as spool,
            tc.tile_pool(name="ps_h", bufs=4, space="PSUM") as ps_h,
            tc.tile_pool(name="ps_a", bufs=2, space="PSUM") as ps_a,
            tc.tile_pool(name="ps_o", bufs=2, space="PSUM") as ps_o,
        ):
            _prio = [0]

            def pdma(q, dst, src):
                inst = q.dma_start(dst, src)
                inst.ins.bass_priority = _prio[0]
                _prio[0] += 1
                return inst

            # warm-up operand (zeros; the 32 warm-up matmuls read it)
            warm_sb = wpool.tile([128, 128], CDT, name="warm")
            if COMPUTE_DT == "float32r":
                nc.vector.memset(warm_sb[:].bitcast(_FP32), 0.0)
            else:
                nc.vector.memset(warm_sb[:], 0.0)

            # ---- DMA issue (priorities pin queue order to program order).
            # First the interleaved wsh/x0 pieces so M1 can start early.
            def xview(s):
                o = s * TCH * D
                return xT_d[o : o + TCH * D].rearrange(
                    "(ko p t) -> p ko t", p=128, t=TCH
                )

            wsh_view = wsh_d.rearrange("(o p) h -> p o h", p=128)
            wsh_ks = [wpool.tile([128, H], CDT, name=f"wshk{k}") for k in range(KD)]
            x0v = xview(0)
            x0 = [xpool.tile([128, TCH], CDT, name=f"x0_{k}") for k in range(KD)]
            for k in range(KD):
                qa = nc.sync if k % 2 == 0 else nc.scalar
                qb = nc.scalar if k % 2 == 0 else nc.sync
                pdma(qa, wsh_ks[k][:], wsh_view[:, k])
                pdma(qb, x0[k][:], x0v[:, k])

            xs = [None] * NSUP
            if NSUP > 1:
                xs[1] = xpool.tile([128, KD, TCH], CDT, name="xs1")
                pdma(nc.sync, xs[1][:], xview(1))
            w1_view = w1_d.rearrange("(o p) f -> p o f", p=128)
            w1_ks = [wpool.tile([128, W1W], CDT, name=f"w1k{k}") for k in range(KH)]
            for k in range(KH):
                pdma(nc.sync, w1_ks[k][:], w1_view[:, k])
            mexp_sb = wpool.tile([128, SKT, PS], CDT, name="mexp")
            pdma(nc.scalar, mexp_sb[:], mexp_d.rearrange("p (s t) -> p s t", t=PS))
            w2_sb = wpool.tile([128, SKT], CDT, name="w2bd")
            pdma(nc.scalar, w2_sb[:], w2_d[:])
            bias_sb = wpool.tile([128, NB], _FP32, name="biases")
            pdma(nc.scalar, bias_sb[:], bias_d[:])
            b2r_sb = wpool.tile([1, BL], _FP32, name="b2row")
            pdma(nc.scalar, b2r_sb[:], b2r_d[:])
            for s in range(2, NSUP):
                xs[s] = xpool.tile([128, KD, TCH], CDT, name=f"xs{s}")
                pdma(nc.sync, xs[s][:], xview(s))

            # ---- warm-up matmuls: heat the PE HAM clock gate during the
            # DMA lead-in (PE is otherwise idle for the first ~9us).
            pw = ps_h.tile([128, TCH], _FP32, tag="ps_h", name="pwarm")
            for _ in range(NWARM):
                nc.tensor.matmul(
                    pw[:, :128], lhsT=warm_sb[:], rhs=warm_sb[:],
                    start=True, stop=True,
                )

            hTs = [None] * NSUP

            def m1_super(s):
                hT = midpool.tile([128, MH, TCH], CDT, tag="hT", name=f"hT{s}")
                hTs[s] = hT
                if s == 0:
                    # k-outer: matmuls start as soon as the first split x
                    # pieces land.
                    phs = [
                        ps_h.tile([128, TCH], _FP32, tag="ps_h", name=f"ph0_{m}")
                        for m in range(MH)
                    ]
                    for k in range(KD):
                        for m in range(MH):
                            nc.tensor.matmul(
                                phs[m][:],
                                lhsT=wsh_ks[k][:, m * 128 : (m + 1) * 128],
                                rhs=x0[k][:],
                                start=(k == 0),
                                stop=(k == KD - 1),
                            )
                    for m in range(MH):
                        nc.scalar.activation(
                            hT[:, m, :], phs[m][:], relu,
                            bias=bias_sb[:, m : m + 1],
                        )
                else:
                    for m in range(MH):
                        ph = ps_h.tile(
                            [128, TCH], _FP32, tag="ps_h", name=f"ph{s}_{m}"
                        )
                        for k in range(KD):
                            nc.tensor.matmul(
                                ph[:],
                                lhsT=wsh_ks[k][:, m * 128 : (m + 1) * 128],
                                rhs=xs[s][:, k, :],
                                start=(k == 0),
                                stop=(k == KD - 1),
                            )
                        nc.scalar.activation(
                            hT[:, m, :], ph[:], relu, bias=bias_sb[:, m : m + 1]
                        )

            def m2m3(p):
                s, q = divmod(p, QP)
                hT = hTs[s]
                t0, t1 = q * PS, (q + 1) * PS
                kt = kt3[p]
                aT = apool.tile([128, kt, PS], CDT, tag="aT", name=f"aT{p}")
                am = apool.tile([128, kt, PS], CDT, tag="am", name=f"am{p}")
                for m in range(kt):
                    pa = ps_a.tile([128, PS], _FP32, tag="ps_a", name=f"pa{p}_{m}")
                    c0 = col0[p] + m * 128
                    for k in range(KH):
                        nc.tensor.matmul(
                            pa[:],
                            lhsT=w1_ks[k][:, c0 : c0 + 128],
                            rhs=hT[:, k, t0:t1],
                            start=(k == 0),
                            stop=(k == KH - 1),
                        )
                    nc.scalar.activation(
                        aT[:, m, :], pa[:], relu,
                        bias=bias_sb[:, MH + boff[p] + m : MH + boff[p] + m + 1],
                    )
                    # fold the routing mask into the activations
                    nc.vector.tensor_mul(
                        am[:, m, :], aT[:, m, :], mexp_sb[:, boff[p] + m, :]
                    )
                po = ps_o.tile([1, PS], _FP32, tag="ps_o", name=f"po{p}")
                for k in range(kt):
                    nc.tensor.matmul(
                        po[:],
                        lhsT=w2_sb[:, boff[p] + k : boff[p] + k + 1],
                        rhs=am[:, k, :],
                        start=(k == 0),
                        stop=(k == kt - 1),
                    )
                g0 = p * PS
                ot = spool.tile([1, PS], _FP32, tag="ot", name=f"ot{p}")
                nc.vector.tensor_add(ot[:], po[:], b2r_sb[:, g0 : g0 + PS])
                nc.gpsimd.dma_start(
                    out_d[g0 : g0 + PS].rearrange("(o t) -> o t", o=1), ot[:]
                )

            # ---- software-pipelined emission: keep one M1 superchunk
            # queued ahead so the tensor engine never starves on the
            # scalar/vector chain of M2/M3.
            m1_super(0)
            if NSUP > 1:
                m1_super(1)
            for p in range(NPOS):
                m2m3(p)
                s, q = divmod(p, QP)
                if q == QP - 1 and s + 2 < NSUP:
                    m1_super(s + 2)

    nc.compile()
    return nc


def get_nc(pattern):
    key = (COMPUTE_DT, tuple(pattern))
    if key not in _cache:
        _cache[key] = _build_nc(tuple(pattern))
    return _cache[key]


def prepare(inputs):
    """Host-side routing/sorting/sharding.

    Returns (pattern, in_maps, tok_order):  out[tok_order] = concat of the
    per-core position-ordered outputs.
    """
    np_dt = _np_in_dtype()
    x = np.asarray(inputs["x"], dtype=np.float32)
    idx = np.asarray(inputs["idx"]).astype(np.int64).reshape(B)
    W_shared = np.asarray(inputs["W_shared"], dtype=np.float32)
    b_shared = np.asarray(inputs["b_shared"], dtype=np.float32).reshape(H)
    W1 = np.asarray(inputs["W1"], dtype=np.float32)
    b1 = np.asarray(inputs["b1"], dtype=np.float32).reshape(E, F)
    W2 = np.asarray(inputs["W2"], dtype=np.float32).reshape(E, F)
    b2 = np.asarray(inputs["b2"], dtype=np.float32).reshape(E)
    send_to = np.asarray(inputs["send_to"]).astype(np.int64)

    perm = np.argsort(idx, kind="stable")
    idx_s = idx[perm]
    routes_s = send_to[idx_s]                      # [B, K] sorted routes
    x_s = x[perm]

    NCH = B // PS                                  # global 512-token chunks
    chex = [np.unique(routes_s[g * PS : (g + 1) * PS]) for g in range(NCH)]
    order = np.argsort([-len(e) for e in chex], kind="stable")
    # position p of core c gets global chunk order[p*N_CORES + c]
    pattern = tuple(
        int(max(len(chex[order[p * N_CORES + c]]) for c in range(N_CORES)))
        for p in range(NPOS)
    )
    kt3, col0, W1W, boff, SKT = _geom(pattern)
    NB = MH + SKT

    wsh = np.ascontiguousarray(W_shared).astype(np_dt)
    TCH = _tch()
    QP = TCH // PS

    in_maps = []
    tok_order_parts = []
    for c in range(N_CORES):
        w1all = np.zeros((H, W1W), dtype=np.float32)
        mexp = np.zeros((128, SKT, PS), dtype=np.float32)
        w2bd = np.zeros((128, SKT), dtype=np.float32)
        biases = np.zeros((128, NB), dtype=np.float32)
        biases[:, :MH] = b_shared.reshape(MH, 128).T
        b2row = np.zeros((1, BL), dtype=np.float32)
        xrows = []
        for p in range(NPOS):
            g = order[p * N_CORES + c]
            toks = perm[g * PS : (g + 1) * PS]
            tok_order_parts.append(toks)
            xrows.append(x_s[g * PS : (g + 1) * PS])
            r = routes_s[g * PS : (g + 1) * PS]    # [PS, K]
            el = chex[g]
            ecp, kt = pattern[p], kt3[p]
            slots = np.full(ecp, -1, dtype=np.int64)
            slots[: len(el)] = el

            b1blk = np.zeros(kt * 128, dtype=np.float32)
            w2blk = np.zeros(kt * 128, dtype=np.float32)
            for j, e in enumerate(slots):
                if e < 0:
                    continue
                w1all[:, col0[p] + j * F : col0[p] + (j + 1) * F] = W1[e]
                b1blk[j * F : (j + 1) * F] = b1[e]
                w2blk[j * F : (j + 1) * F] = W2[e]
            biases[:, MH + boff[p] : MH + boff[p] + kt] = (
                b1blk.reshape(kt, 128).T
            )
            w2bd[:, boff[p] : boff[p] + kt] = w2blk.reshape(kt, 128).T

            # routing mask per slot, expanded to the f' rows of each slot
            mrow = np.zeros((ecp, PS), dtype=np.float32)
            for kk in range(r.shape[1]):
                mrow += (slots[:, None] == r[None, :, kk]) / r.shape[1]
            mflat = np.zeros((kt * 128, PS), dtype=np.float32)
            for j in range(ecp):
                mflat[j * F : (j + 1) * F] = mrow[j]
            mexp[:, boff[p] : boff[p] + kt, :] = (
                mflat.reshape(kt, 128, PS).transpose(1, 0, 2)
            )
            b2row[0, p * PS : (p + 1) * PS] = b2[r].mean(axis=1)

        # xT layout: per superchunk (QP positions), k-tile-major
        parts = []
        for s in range(BL // TCH):
            xsup = np.concatenate(xrows[s * QP : (s + 1) * QP], axis=0)  # [TCH, D]
            parts.append(
                np.ascontiguousarray(xsup.T).reshape(KD, 128, TCH).ravel()
            )
        xT = np.concatenate(parts).astype(np_dt)

        in_maps.append(
            {
                "xT": xT,
                "wsh": wsh,
                "w1all": w1all.astype(np_dt),
                "mexp": np.ascontiguousarray(
                    mexp.reshape(128, SKT * PS)
                ).astype(np_dt),
                "w2bd": w2bd.astype(np_dt),
                "biases": biases,
                "b2row": b2row,
            }
        )
    tok_order = np.concatenate(tok_order_parts)
    return pattern, in_maps, tok_order


def kernel(**inputs) -> np.ndarray:
    pattern, in_maps, tok_order = prepare(inputs)
    nc = get_nc(pattern)
    res = run_bass_kernel_spmd(nc, in_maps, list(range(N_CORES)))
    out_sorted = np.concatenate([res.results[c]["out"] for c in range(N_CORES)])
    out = np.empty(B, dtype=np.float32)
    out[tok_order] = out_sorted
    return out.reshape(B, 1)


# revision 7
# speedup vs baseline: 1.3205x; 1.0842x over previous
"""Trainium2 Bass kernel for the MoE-routing module (v3).

Computation (B=32768, D=1024, H=512, F=100, E=16, K=2):
    h   = relu(x @ W_shared + b_shared)                  [B, H]
    a   = relu(einsum('bh,ehf', h, W1) + b1)             [B, E, F]
    o   = einsum('bef,efo', a, W2) + b2                  [B, E, 1]
    out = mean over the K routed experts of o[b, send_to[idx[b]]]

Strategy:
  * Host sorts tokens by head id, cuts the sorted batch into 64 chunks of
    512 tokens.  A chunk touches 2 experts (3 when it straddles a head-id
    boundary; there are <=15 such chunks).  Chunks are dealt to the 8
    cores so every core sees the same per-position expert-slot pattern
    (typically [3,3,2,2,2,2,2,2]) -> a single SPMD program.
  * Per-core compute, features on SBUF partitions throughout (bf16):
      M1: hT[h, t]  = relu(W_shared.T @ xT)    512-token chunks
      M2: aT[f',t]  = relu(W1sel.T @ hT)       f' = chunk-local slot*F + f
      mask fold:  am = aT * mexp   (routing mask expanded to f' rows)
      M3: out[t]    = sum_k w2col_k.T @ am_k   (single-partition result)
    Folding the mask into the activations removes the select-matmul and
    its PSUM bank: PSUM = 4 (M1) + 2 (M2) + 2 (M3) banks.
  * bf16 operands: half the HBM bytes of fp32, same PE column rate, and
    FWL (fast weight load, disabled for fp32) hides LDWEIGHTS.
  * DMA data is laid out partition-major so every descriptor is 128
    contiguous multi-KB lines (cheap to issue); big transfers go on the
    Sync queue in need-order, small early items (wsh half, x0 pieces) on
    the Scalar queue before the RELUs.  Zero biases are elided entirely
    so no activation ever waits on a bias DMA.
  * ~32 warm-up matmuls run during the DMA lead-in so the PE HAM clock
    gate is already at 2.4 GHz when real work arrives; M1 runs three
    chunks ahead of M2 so the tensor queue never head-blocks on the
    scalar/vector chain.
"""

import os

import numpy as np

import concourse.mybir as mybir
from concourse import bacc
from concourse.bass_utils import run_bass_kernel_spmd
from concourse.tile import TileContext

B, D, H, F, E, TOPK = 32768, 1024, 512, 100, 16, 2
N_CORES = 8
BL = B // N_CORES          # tokens per core (4096)
PS = 512                   # tokens per chunk (= matmul moving width)
NPOS = BL // PS            # chunk positions per core (8)
MH = H // 128              # M1 output tiles (4)
KD = D // 128              # M1 contraction tiles (8)
KH = H // 128              # M2 contraction tiles (4)
NWARM = int(os.environ.get("KERNEL_WARMUP", "32"))
WARM_MEMSET = os.environ.get("KERNEL_WARM_MEMSET", "1") == "1"

# Compute dtype for the matmul stages: "float32", "float32r", or "bfloat16"
COMPUTE_DT = os.environ.get("KERNEL_DT", "bfloat16")

_FP32 = mybir.dt.float32
_cache = {}


def _np_in_dtype():
    import ml_dtypes

    return ml_dtypes.bfloat16 if COMPUTE_DT == "bfloat16" else np.float32


def _geom(pattern):
    """Derived geometry for a per-position expert-slot pattern."""
    kt3 = [(ec * F + 127) // 128 for ec in pattern]
    col0, c = [], 0
    for ec in pattern:
        col0.append(c)
        c += ec * F
    w1w = max(col0[p] + kt3[p] * 128 for p in range(len(pattern)))
    boff, b = [], 0
    for k in kt3:
        boff.append(b)
        b += k
    skt = b
    return kt3, col0, w1w, boff, skt


def _build_nc(key):
    """Build the SPMD program for (pattern, zero_bias, zero_b2)."""
    pattern, zero_bias, zero_b2 = key
    CDT = getattr(mybir.dt, COMPUTE_DT)
    kt3, col0, W1W, boff, SKT = _geom(pattern)
    NB = MH + SKT
    NA = boff[NPOS // 2]          # mexp blocks in the first half

    nc = bacc.Bacc("TRN2", target_bir_lowering=False, num_devices=N_CORES)

    xT_d = nc.declare_dram_parameter("xT", [D * BL], CDT, isOutput=False)
    wsh_d = nc.declare_dram_parameter("wsh", [128, KD * H], CDT, isOutput=False)
    w1_d = nc.declare_dram_parameter("w1all", [H, W1W], CDT, isOutput=False)
    mexp_d = nc.declare_dram_parameter("mexp", [128, SKT * PS], CDT, isOutput=False)
    w2_d = nc.declare_dram_parameter("w2bd", [128, SKT], CDT, isOutput=False)
    if not zero_bias:
        bias_d = nc.declare_dram_parameter("biases", [128, NB], _FP32, isOutput=False)
    if not zero_b2:
        b2r_d = nc.declare_dram_parameter("b2row", [1, BL], _FP32, isOutput=False)
    out_d = nc.declare_dram_parameter("out", [BL], _FP32, isOutput=True)

    relu = mybir.ActivationFunctionType.Relu

    with TileContext(nc) as tc:
        with (
            tc.tile_pool(name="weights", bufs=1) as wpool,
            tc.tile_pool(name="xin", bufs=1) as xpool,
            tc.tile_pool(name="mid", bufs=4) as midpool,
            tc.tile_pool(name="act", bufs=2) as apool,
            tc.tile_pool(name="small", bufs=3) as spool,
            tc.tile_pool(name="ps_h", bufs=4, space="PSUM") as ps_h,
            tc.tile_pool(name="ps_a", bufs=2, space="PSUM") as ps_a,
            tc.tile_pool(name="ps_o", bufs=2, space="PSUM") as ps_o,
        ):
            _prio = [0]

            def pdma(q, dst, src):
                inst = q.dma_start(dst, src)
                inst.ins.bass_priority = _prio[0]
                _prio[0] += 1
                return inst

            warm_sb = wpool.tile([128, 128], CDT, name="warm")
            if WARM_MEMSET:
                if COMPUTE_DT == "float32r":
                    nc.vector.memset(warm_sb[:].bitcast(_FP32), 0.0)
                else:
                    nc.vector.memset(warm_sb[:], 0.0)

            # ---- DMA issue.  All sources are partition-major: every
            # descriptor is 128 contiguous multi-KB lines.  bass_priority
            # pins each queue's order to program order.
            def xview(s):
                o = s * PS * D
                return xT_d[o : o + PS * D].rearrange(
                    "(p ko t) -> p ko t", p=128, t=PS
                )

            wsh_view = wsh_d.rearrange("p (o h) -> p o h", h=H)
            wsh_a = wpool.tile([128, KD // 2, H], CDT, name="wsh_a")
            wsh_b = wpool.tile([128, KD - KD // 2, H], CDT, name="wsh_b")
            x0v = xview(0)
            x0 = [xpool.tile([128, PS], CDT, name=f"x0_{k}") for k in range(KD)]

            # scalar queue: small, early-needed pieces, then RELUs.
            if not zero_bias:
                bias_sb = wpool.tile([128, NB], _FP32, name="biases")
                pdma(nc.scalar, bias_sb[:], bias_d[:])
            if not zero_b2:
                b2r_sb = wpool.tile([1, BL], _FP32, name="b2row")
                pdma(nc.scalar, b2r_sb[:], b2r_d[:])
            pdma(nc.scalar, wsh_b[:], wsh_view[:, KD // 2 :])
            # sync queue: everything else, in first-need order.
            pdma(nc.sync, wsh_a[:], wsh_view[:, : KD // 2])
            for k in range(0, KD, 2):
                pdma(nc.sync, x0[k][:], x0v[:, k])
                pdma(nc.scalar, x0[k + 1][:], x0v[:, k + 1])

            def wsh_k(k, m):
                t = wsh_a if k < KD // 2 else wsh_b
                return t[:, k % (KD // 2), m * 128 : (m + 1) * 128]

            xs = [None] * NPOS
            w1_view = w1_d.rearrange("(o p) f -> p o f", p=128)
            w1_ks = [wpool.tile([128, W1W], CDT, name=f"w1k{k}") for k in range(KH)]
            mexp_view = mexp_d.rearrange("p (s t) -> p s t", t=PS)
            mexp_a = wpool.tile([128, NA, PS], CDT, name="mexp_a")
            mexp_b = wpool.tile([128, SKT - NA, PS], CDT, name="mexp_b")
            w2_sb = wpool.tile([128, SKT], CDT, name="w2bd")

            def xdma(s):
                xs[s] = xpool.tile([128, KD, PS], CDT, name=f"xs{s}")
                pdma(nc.sync, xs[s][:], xview(s))

            xdma(1)
            pdma(nc.sync, w1_ks[0][:], w1_view[:, 0])
            pdma(nc.sync, w1_ks[1][:], w1_view[:, 1])
            xdma(2)
            pdma(nc.sync, w1_ks[2][:], w1_view[:, 2])
            pdma(nc.sync, w1_ks[3][:], w1_view[:, 3])
            pdma(nc.sync, mexp_a[:], mexp_view[:, :NA])
            pdma(nc.sync, w2_sb[:], w2_d[:])
            xdma(3)
            pdma(nc.sync, mexp_b[:], mexp_view[:, NA:])
            for s in range(4, NPOS):
                xdma(s)

            # ---- warm-up matmuls: heat the PE HAM clock gate during the
            # DMA lead-in (PE is otherwise idle for the first ~9us).
            pw = ps_h.tile([128, PS], _FP32, tag="ps_h", name="pwarm")
            for _ in range(NWARM):
                nc.tensor.matmul(
                    pw[:, :128], lhsT=warm_sb[:], rhs=warm_sb[:],
                    start=True, stop=True,
                )

            hTs = [None] * NPOS

            def m1_chunk(s):
                hT = midpool.tile([128, MH, PS], CDT, tag="hT", name=f"hT{s}")
                hTs[s] = hT

                def act(m, ph):
                    if zero_bias:
                        nc.scalar.activation(hT[:, m, :], ph[:], relu)
                    else:
                        nc.scalar.activation(
                            hT[:, m, :], ph[:], relu, bias=bias_sb[:, m : m + 1]
                        )

                if s == 0:
                    # k-outer: matmuls start as soon as the first split x
                    # pieces land.
                    phs = [
                        ps_h.tile([128, PS], _FP32, tag="ps_h", name=f"ph0_{m}")
                        for m in range(MH)
                    ]
                    for k in range(KD):
                        for m in range(MH):
                            nc.tensor.matmul(
                                phs[m][:],
                                lhsT=wsh_k(k, m),
                                rhs=x0[k][:],
                                start=(k == 0),
                                stop=(k == KD - 1),
                            )
                    for m in range(MH):
                        act(m, phs[m])
                else:
                    for m in range(MH):
                        ph = ps_h.tile(
                            [128, PS], _FP32, tag="ps_h", name=f"ph{s}_{m}"
                        )
                        for k in range(KD):
                            nc.tensor.matmul(
                                ph[:],
                                lhsT=wsh_k(k, m),
                                rhs=xs[s][:, k, :],
                                start=(k == 0),
                                stop=(k == KD - 1),
                            )
                        act(m, ph)

            def m2m3(p):
                hT = hTs[p]
                kt = kt3[p]
                if boff[p] < NA:
                    mx, mo = mexp_a, boff[p]
                else:
                    mx, mo = mexp_b, boff[p] - NA
                aT = apool.tile([128, kt, PS], CDT, tag="aT", name=f"aT{p}")
                am = apool.tile([128, kt, PS], CDT, tag="am", name=f"am{p}")
                for m in range(kt):
                    pa = ps_a.tile([128, PS], _FP32, tag="ps_a", name=f"pa{p}_{m}")
                    c0 = col0[p] + m * 128
                    for k in range(KH):
                        nc.tensor.matmul(
                            pa[:],
                            lhsT=w1_ks[k][:, c0 : c0 + 128],
                            rhs=hT[:, k, :],
                            start=(k == 0),
                            stop=(k == KH - 1),
                        )
                    if zero_bias:
                        nc.scalar.activation(aT[:, m, :], pa[:], relu)
                    else:
                        nc.scalar.activation(
                            aT[:, m, :], pa[:], relu,
                            bias=bias_sb[
                                :, MH + boff[p] + m : MH + boff[p] + m + 1
                            ],
                        )
                    # fold the routing mask into the activations
                    nc.vector.tensor_mul(
                        am[:, m, :], aT[:, m, :], mx[:, mo + m, :]
                    )
                po = ps_o.tile([1, PS], _FP32, tag="ps_o", name=f"po{p}")
                for k in range(kt):
                    nc.tensor.matmul(
                        po[:],
                        lhsT=w2_sb[:, boff[p] + k : boff[p] + k + 1],
                        rhs=am[:, k, :],
                        start=(k == 0),
                        stop=(k == kt - 1),
                    )
                g0 = p * PS
                ot = spool.tile([1, PS], _FP32, tag="ot", name=f"ot{p}")
                if zero_b2:
                    nc.vector.tensor_copy(ot[:], po[:])
                else:
                    nc.vector.tensor_add(ot[:], po[:], b2r_sb[:, g0 : g0 + PS])
                nc.gpsimd.dma_start(
                    out_d[g0 : g0 + PS].rearrange("(o t) -> o t", o=1), ot[:]
                )

            # ---- software-pipelined emission: keep up to three M1 chunks
            # queued ahead so the tensor engine never starves on late DMAs
            # or the scalar/vector chain of M2/M3.
            m1_chunk(0)
            m1_chunk(1)
            m1_chunk(2)
            for p in range(NPOS):
                m2m3(p)
                if p + 3 < NPOS:
                    m1_chunk(p + 3)

    nc.compile()
    return nc


def get_nc(key):
    ckey = (COMPUTE_DT, key)
    if ckey not in _cache:
        _cache[ckey] = _build_nc(key)
    return _cache[ckey]


def prepare(inputs):
    """Host-side routing/sorting/sharding.

    Returns (key, in_maps, tok_order):  out[tok_order] = concat of the
    per-core position-ordered outputs.
    """
    np_dt = _np_in_dtype()
    x = np.asarray(inputs["x"], dtype=np.float32)
    idx = np.asarray(inputs["idx"]).astype(np.int64).reshape(B)
    W_shared = np.asarray(inputs["W_shared"], dtype=np.float32)
    b_shared = np.asarray(inputs["b_shared"], dtype=np.float32).reshape(H)
    W1 = np.asarray(inputs["W1"], dtype=np.float32)
    b1 = np.asarray(inputs["b1"], dtype=np.float32).reshape(E, F)
    W2 = np.asarray(inputs["W2"], dtype=np.float32).reshape(E, F)
    b2 = np.asarray(inputs["b2"], dtype=np.float32).reshape(E)
    send_to = np.asarray(inputs["send_to"]).astype(np.int64)

    zero_bias = not (np.any(b_shared) or np.any(b1))
    zero_b2 = not np.any(b2)

    perm = np.argsort(idx, kind="stable")
    idx_s = idx[perm]
    routes_s = send_to[idx_s]                      # [B, K] sorted routes
    x_s = x[perm]

    NCH = B // PS                                  # global 512-token chunks
    chex = [np.unique(routes_s[g * PS : (g + 1) * PS]) for g in range(NCH)]
    order = np.argsort([-len(e) for e in chex], kind="stable")
    # position p of core c gets global chunk order[p*N_CORES + c]
    pattern = tuple(
        int(max(len(chex[order[p * N_CORES + c]]) for c in range(N_CORES)))
        for p in range(NPOS)
    )
    kt3, col0, W1W, boff, SKT = _geom(pattern)
    NB = MH + SKT

    # partition-major W_shared: [128, KD*H], row p holds its k-tiles
    wsh = np.ascontiguousarray(
        W_shared.reshape(KD, 128, H).transpose(1, 0, 2).reshape(128, KD * H)
    ).astype(np_dt)

    key = (pattern, zero_bias, zero_b2)
    in_maps = []
    tok_order_parts = []
    for c in range(N_CORES):
        w1all = np.zeros((H, W1W), dtype=np.float32)
        mexp = np.zeros((128, SKT, PS), dtype=np.float32)
        w2bd = np.zeros((128, SKT), dtype=np.float32)
        biases = np.zeros((128, NB), dtype=np.float32)
        biases[:, :MH] = b_shared.reshape(MH, 128).T
        b2row = np.zeros((1, BL), dtype=np.float32)
        xparts = []
        for p in range(NPOS):
            g = order[p * N_CORES + c]
            toks = perm[g * PS : (g + 1) * PS]
            tok_order_parts.append(toks)
            # partition-major x chunk: [128, KD, PS]
            xc = x_s[g * PS : (g + 1) * PS]                    # [PS, D]
            xparts.append(
                np.ascontiguousarray(xc.T)
                .reshape(KD, 128, PS)
                .transpose(1, 0, 2)
                .ravel()
            )
            r = routes_s[g * PS : (g + 1) * PS]                # [PS, K]
            el = chex[g]
            ecp, kt = pattern[p], kt3[p]
            slots = np.full(ecp, -1, dtype=np.int64)
            slots[: len(el)] = el

            b1blk = np.zeros(kt * 128, dtype=np.float32)
            w2blk = np.zeros(kt * 128, dtype=np.float32)
            for j, e in enumerate(slots):
                if e < 0:
                    continue
                w1all[:, col0[p] + j * F : col0[p] + (j + 1) * F] = W1[e]
                b1blk[j * F : (j + 1) * F] = b1[e]
                w2blk[j * F : (j + 1) * F] = W2[e]
            biases[:, MH + boff[p] : MH + boff[p] + kt] = (
                b1blk.reshape(kt, 128).T
            )
            w2bd[:, boff[p] : boff[p] + kt] = w2blk.reshape(kt, 128).T

            # routing mask per slot, expanded to the f' rows of each slot
            mrow = np.zeros((ecp, PS), dtype=np.float32)
            for kk in range(r.shape[1]):
                mrow += (slots[:, None] == r[None, :, kk]) / r.shape[1]
            mflat = np.zeros((kt * 128, PS), dtype=np.float32)
            for j in range(ecp):
                mflat[j * F : (j + 1) * F] = mrow[j]
            mexp[:, boff[p] : boff[p] + kt, :] = (
                mflat.reshape(kt, 128, PS).transpose(1, 0, 2)
            )
            b2row[0, p * PS : (p + 1) * PS] = b2[r].mean(axis=1)

        xT = np.concatenate(xparts).astype(np_dt)
        im = {
            "xT": xT,
            "wsh": wsh,
            "w1all": w1all.astype(np_dt),
            "mexp": np.ascontiguousarray(mexp.reshape(128, SKT * PS)).astype(
                np_dt
            ),
            "w2bd": w2bd.astype(np_dt),
        }
        if not zero_bias:
            im["biases"] = biases
        if not zero_b2:
            im["b2row"] = b2row
        in_maps.append(im)
    tok_order = np.concatenate(tok_order_parts)
    return key, in_maps, tok_order


def kernel(**inputs) -> np.ndarray:
    key, in_maps, tok_order = prepare(inputs)
    nc = get_nc(key)
    res = run_bass_kernel_spmd(nc, in_maps, list(range(N_CORES)))
    out_sorted = np.concatenate([res.results[c]["out"] for c in range(N_CORES)])
    out = np.empty(B, dtype=np.float32)
    out[tok_order] = out_sorted
    return out.reshape(B, 1)


# revision 9
# speedup vs baseline: 1.3424x; 1.0166x over previous
"""Trainium2 Bass kernel for the MoE-routing module (v3).

Computation (B=32768, D=1024, H=512, F=100, E=16, K=2):
    h   = relu(x @ W_shared + b_shared)                  [B, H]
    a   = relu(einsum('bh,ehf', h, W1) + b1)             [B, E, F]
    o   = einsum('bef,efo', a, W2) + b2                  [B, E, 1]
    out = mean over the K routed experts of o[b, send_to[idx[b]]]

Strategy:
  * Host sorts tokens by head id, cuts the sorted batch into 64 chunks of
    512 tokens.  A chunk touches 2 experts (3 when it straddles a head-id
    boundary; there are <=15 such chunks).  Chunks are dealt to the 8
    cores so every core sees the same per-position expert-slot pattern
    (typically [3,3,2,2,2,2,2,2]) -> a single SPMD program.
  * Per-core compute, features on SBUF partitions throughout (bf16):
      M1: hT[h, t]  = relu(W_shared.T @ xT)    512-token chunks
      M2: aT[f',t]  = relu(W1sel.T @ hT)       f' = chunk-local slot*F + f
      mask fold:  am = aT * mexp   (routing mask expanded to f' rows)
      M3: out[t]    = sum_k w2col_k.T @ am_k   (single-partition result)
    Folding the mask into the activations removes the select-matmul and
    its PSUM bank: PSUM = 4 (M1) + 2 (M2) + 2 (M3) banks.
  * bf16 operands: half the HBM bytes of fp32, same PE column rate, and
    FWL (fast weight load, disabled for fp32) hides LDWEIGHTS.
  * DMA data is laid out partition-major so every descriptor is 128
    contiguous multi-KB lines (cheap to issue); big transfers go on the
    Sync queue in need-order, small early items (wsh half, x0 pieces) on
    the Scalar queue before the RELUs.  Zero biases are elided entirely
    so no activation ever waits on a bias DMA.
  * ~32 warm-up matmuls run during the DMA lead-in so the PE HAM clock
    gate is already at 2.4 GHz when real work arrives; M1 runs three
    chunks ahead of M2 so the tensor queue never head-blocks on the
    scalar/vector chain.
"""

import os

import numpy as np

import concourse.mybir as mybir
from concourse import bacc
from concourse.bass_utils import run_bass_kernel_spmd
from concourse.tile import TileContext

B, D, H, F, E, TOPK = 32768, 1024, 512, 100, 16, 2
N_CORES = 8
BL = B // N_CORES          # tokens per core (4096)
PS = 512                   # tokens per chunk (= matmul moving width)
NPOS = BL // PS            # chunk positions per core (8)
MH = H // 128              # M1 output tiles (4)
KD = D // 128              # M1 contraction tiles (8)
KH = H // 128              # M2 contraction tiles (4)
NWARM = int(os.environ.get("KERNEL_WARMUP", "32"))
WARM_MEMSET = os.environ.get("KERNEL_WARM_MEMSET", "1") == "1"

# Compute dtype for the matmul stages: "float32", "float32r", or "bfloat16"
COMPUTE_DT = os.environ.get("KERNEL_DT", "bfloat16")

_FP32 = mybir.dt.float32
_cache = {}


def _np_in_dtype():
    import ml_dtypes

    return ml_dtypes.bfloat16 if COMPUTE_DT == "bfloat16" else np.float32


def _geom(pattern):
    """Derived geometry for a per-position expert-slot pattern."""
    kt3 = [(ec * F + 127) // 128 for ec in pattern]
    col0, c = [], 0
    for ec in pattern:
        col0.append(c)
        c += ec * F
    w1w = max(col0[p] + kt3[p] * 128 for p in range(len(pattern)))
    boff, b = [], 0
    for k in kt3:
        boff.append(b)
        b += k
    skt = b
    return kt3, col0, w1w, boff, skt


def _build_nc(key):
    """Build the SPMD program for (pattern, zero_bias, zero_b2)."""
    pattern, zero_bias, zero_b2 = key
    CDT = getattr(mybir.dt, COMPUTE_DT)
    kt3, col0, W1W, boff, SKT = _geom(pattern)
    NB = MH + SKT
    NA = boff[NPOS // 2]          # mexp blocks in the first half

    nc = bacc.Bacc("TRN2", target_bir_lowering=False, num_devices=N_CORES)

    xT_d = nc.declare_dram_parameter("xT", [D * BL], CDT, isOutput=False)
    wsh_d = nc.declare_dram_parameter("wsh", [128, KD * H], CDT, isOutput=False)
    w1_d = nc.declare_dram_parameter("w1all", [H, W1W], CDT, isOutput=False)
    mexp_d = nc.declare_dram_parameter("mexp", [128, SKT * PS], CDT, isOutput=False)
    w2_d = nc.declare_dram_parameter("w2bd", [128, SKT], CDT, isOutput=False)
    if not zero_bias:
        bias_d = nc.declare_dram_parameter("biases", [128, NB], _FP32, isOutput=False)
    if not zero_b2:
        b2r_d = nc.declare_dram_parameter("b2row", [1, BL], _FP32, isOutput=False)
    out_d = nc.declare_dram_parameter("out", [BL], _FP32, isOutput=True)

    relu = mybir.ActivationFunctionType.Relu

    with TileContext(nc) as tc:
        with (
            tc.tile_pool(name="weights", bufs=1) as wpool,
            tc.tile_pool(name="xin", bufs=1) as xpool,
            tc.tile_pool(name="mid", bufs=4) as midpool,
            tc.tile_pool(name="act", bufs=2) as apool,
            tc.tile_pool(name="small", bufs=3) as spool,
            tc.tile_pool(name="ps_h", bufs=4, space="PSUM") as ps_h,
            tc.tile_pool(name="ps_a", bufs=2, space="PSUM") as ps_a,
            tc.tile_pool(name="ps_o", bufs=2, space="PSUM") as ps_o,
        ):
            _prio = [0]

            def pdma(q, dst, src):
                inst = q.dma_start(dst, src)
                inst.ins.bass_priority = _prio[0]
                _prio[0] += 1
                return inst

            warm_sb = wpool.tile([128, 128], CDT, name="warm")
            if WARM_MEMSET:
                if COMPUTE_DT == "float32r":
                    nc.vector.memset(warm_sb[:].bitcast(_FP32), 0.0)
                else:
                    nc.vector.memset(warm_sb[:], 0.0)

            # ---- DMA issue.  All sources are partition-major: every
            # descriptor is 128 contiguous multi-KB lines.  bass_priority
            # pins each queue's order to program order.
            def xview(s):
                o = s * PS * D
                return xT_d[o : o + PS * D].rearrange(
                    "(p ko t) -> p ko t", p=128, t=PS
                )

            wsh_view = wsh_d.rearrange("p (o h) -> p o h", h=H)
            wsh_ks = [wpool.tile([128, H], CDT, name=f"wshk{k}") for k in range(KD)]
            x0v = xview(0)
            x0 = [xpool.tile([128, PS], CDT, name=f"x0_{k}") for k in range(KD)]

            # small, early-needed pieces first; wsh/x0 interleaved in the
            # k-order chunk 0 consumes them, split across both queues.
            if not zero_bias:
                bias_sb = wpool.tile([128, NB], _FP32, name="biases")
                pdma(nc.scalar, bias_sb[:], bias_d[:])
            if not zero_b2:
                b2r_sb = wpool.tile([1, BL], _FP32, name="b2row")
                pdma(nc.scalar, b2r_sb[:], b2r_d[:])
            for k in range(0, KD, 2):
                pdma(nc.sync, wsh_ks[k][:], wsh_view[:, k])
                pdma(nc.sync, x0[k][:], x0v[:, k])
                pdma(nc.scalar, wsh_ks[k + 1][:], wsh_view[:, k + 1])
                pdma(nc.scalar, x0[k + 1][:], x0v[:, k + 1])

            def wsh_k(k, m):
                return wsh_ks[k][:, m * 128 : (m + 1) * 128]

            xs = [None] * NPOS
            w1_view = w1_d.rearrange("(o p) f -> p o f", p=128)
            w1_ks = [wpool.tile([128, W1W], CDT, name=f"w1k{k}") for k in range(KH)]
            mexp_view = mexp_d.rearrange("p (s t) -> p s t", t=PS)
            mexp_a = wpool.tile([128, NA, PS], CDT, name="mexp_a")
            mexp_b = wpool.tile([128, SKT - NA, PS], CDT, name="mexp_b")
            w2_sb = wpool.tile([128, SKT], CDT, name="w2bd")

            def xdma(s):
                xs[s] = xpool.tile([128, KD, PS], CDT, name=f"xs{s}")
                pdma(nc.sync, xs[s][:], xview(s))

            xdma(1)
            pdma(nc.sync, w1_ks[0][:], w1_view[:, 0])
            pdma(nc.sync, w1_ks[1][:], w1_view[:, 1])
            xdma(2)
            pdma(nc.sync, w1_ks[2][:], w1_view[:, 2])
            pdma(nc.sync, w1_ks[3][:], w1_view[:, 3])
            pdma(nc.sync, mexp_a[:], mexp_view[:, :NA])
            pdma(nc.sync, w2_sb[:], w2_d[:])
            xdma(3)
            pdma(nc.sync, mexp_b[:], mexp_view[:, NA:])
            for s in range(4, NPOS):
                xdma(s)

            # ---- warm-up matmuls: heat the PE HAM clock gate during the
            # DMA lead-in (PE is otherwise idle for the first ~9us).
            pw = ps_h.tile([128, PS], _FP32, tag="ps_h", name="pwarm")
            for _ in range(NWARM):
                nc.tensor.matmul(
                    pw[:, :128], lhsT=warm_sb[:], rhs=warm_sb[:],
                    start=True, stop=True,
                )

            hTs = [None] * NPOS

            def m1_chunk(s):
                hT = midpool.tile([128, MH, PS], CDT, tag="hT", name=f"hT{s}")
                hTs[s] = hT

                def act(m, ph):
                    if zero_bias:
                        nc.scalar.activation(hT[:, m, :], ph[:], relu)
                    else:
                        nc.scalar.activation(
                            hT[:, m, :], ph[:], relu, bias=bias_sb[:, m : m + 1]
                        )

                if s == 0:
                    # k-outer: matmuls start as soon as the first split x
                    # pieces land.
                    phs = [
                        ps_h.tile([128, PS], _FP32, tag="ps_h", name=f"ph0_{m}")
                        for m in range(MH)
                    ]
                    for k in range(KD):
                        for m in range(MH):
                            nc.tensor.matmul(
                                phs[m][:],
                                lhsT=wsh_k(k, m),
                                rhs=x0[k][:],
                                start=(k == 0),
                                stop=(k == KD - 1),
                            )
                    for m in range(MH):
                        act(m, phs[m])
                else:
                    for m in range(MH):
                        ph = ps_h.tile(
                            [128, PS], _FP32, tag="ps_h", name=f"ph{s}_{m}"
                        )
                        for k in range(KD):
                            nc.tensor.matmul(
                                ph[:],
                                lhsT=wsh_k(k, m),
                                rhs=xs[s][:, k, :],
                                start=(k == 0),
                                stop=(k == KD - 1),
                            )
                        act(m, ph)

            ams = [None] * NPOS

            def m2(p):
                hT = hTs[p]
                kt = kt3[p]
                if boff[p] < NA:
                    mx, mo = mexp_a, boff[p]
                else:
                    mx, mo = mexp_b, boff[p] - NA
                aT = apool.tile([128, kt, PS], CDT, tag="aT", name=f"aT{p}")
                am = apool.tile([128, kt, PS], CDT, tag="am", name=f"am{p}")
                ams[p] = am
                for m in range(kt):
                    pa = ps_a.tile([128, PS], _FP32, tag="ps_a", name=f"pa{p}_{m}")
                    c0 = col0[p] + m * 128
                    for k in range(KH):
                        nc.tensor.matmul(
                            pa[:],
                            lhsT=w1_ks[k][:, c0 : c0 + 128],
                            rhs=hT[:, k, :],
                            start=(k == 0),
                            stop=(k == KH - 1),
                        )
                    if zero_bias:
                        nc.scalar.activation(aT[:, m, :], pa[:], relu)
                    else:
                        nc.scalar.activation(
                            aT[:, m, :], pa[:], relu,
                            bias=bias_sb[
                                :, MH + boff[p] + m : MH + boff[p] + m + 1
                            ],
                        )
                    # fold the routing mask into the activations
                    nc.vector.tensor_mul(
                        am[:, m, :], aT[:, m, :], mx[:, mo + m, :]
                    )

            def m3out(p):
                kt = kt3[p]
                am = ams[p]
                po = ps_o.tile([1, PS], _FP32, tag="ps_o", name=f"po{p}")
                for k in range(kt):
                    nc.tensor.matmul(
                        po[:],
                        lhsT=w2_sb[:, boff[p] + k : boff[p] + k + 1],
                        rhs=am[:, k, :],
                        start=(k == 0),
                        stop=(k == kt - 1),
                    )
                g0 = p * PS
                ot = spool.tile([1, PS], _FP32, tag="ot", name=f"ot{p}")
                if zero_b2:
                    nc.vector.tensor_copy(ot[:], po[:])
                else:
                    nc.vector.tensor_add(ot[:], po[:], b2r_sb[:, g0 : g0 + PS])
                nc.gpsimd.dma_start(
                    out_d[g0 : g0 + PS].rearrange("(o t) -> o t", o=1), ot[:]
                )

            # ---- software-pipelined emission: M1 runs three chunks ahead
            # of M2, and each M3 trails its M2 by one position, so the
            # tensor queue never waits on the scalar RELU -> vector mask
            # chain or on late DMAs.
            m1_chunk(0)
            m1_chunk(1)
            m1_chunk(2)
            for p in range(NPOS):
                m2(p)
                if p >= 1:
                    m3out(p - 1)
                if p + 3 < NPOS:
                    m1_chunk(p + 3)
            m3out(NPOS - 1)

    nc.compile()
    return nc


def get_nc(key):
    ckey = (COMPUTE_DT, key)
    if ckey not in _cache:
        _cache[ckey] = _build_nc(key)
    return _cache[ckey]


def prepare(inputs):
    """Host-side routing/sorting/sharding.

    Returns (key, in_maps, tok_order):  out[tok_order] = concat of the
    per-core position-ordered outputs.
    """
    np_dt = _np_in_dtype()
    x = np.asarray(inputs["x"], dtype=np.float32)
    idx = np.asarray(inputs["idx"]).astype(np.int64).reshape(B)
    W_shared = np.asarray(inputs["W_shared"], dtype=np.float32)
    b_shared = np.asarray(inputs["b_shared"], dtype=np.float32).reshape(H)
    W1 = np.asarray(inputs["W1"], dtype=np.float32)
    b1 = np.asarray(inputs["b1"], dtype=np.float32).reshape(E, F)
    W2 = np.asarray(inputs["W2"], dtype=np.float32).reshape(E, F)
    b2 = np.asarray(inputs["b2"], dtype=np.float32).reshape(E)
    send_to = np.asarray(inputs["send_to"]).astype(np.int64)

    zero_bias = not (np.any(b_shared) or np.any(b1))
    zero_b2 = not np.any(b2)

    perm = np.argsort(idx, kind="stable")
    idx_s = idx[perm]
    routes_s = send_to[idx_s]                      # [B, K] sorted routes
    x_s = x[perm]

    NCH = B // PS                                  # global 512-token chunks
    chex = [np.unique(routes_s[g * PS : (g + 1) * PS]) for g in range(NCH)]
    order = np.argsort([-len(e) for e in chex], kind="stable")
    # position p of core c gets global chunk order[p*N_CORES + c]
    pattern = tuple(
        int(max(len(chex[order[p * N_CORES + c]]) for c in range(N_CORES)))
        for p in range(NPOS)
    )
    kt3, col0, W1W, boff, SKT = _geom(pattern)
    NB = MH + SKT

    # partition-major W_shared: [128, KD*H], row p holds its k-tiles
    wsh = np.ascontiguousarray(
        W_shared.reshape(KD, 128, H).transpose(1, 0, 2).reshape(128, KD * H)
    ).astype(np_dt)

    key = (pattern, zero_bias, zero_b2)
    in_maps = []
    tok_order_parts = []
    for c in range(N_CORES):
        w1all = np.zeros((H, W1W), dtype=np.float32)
        mexp = np.zeros((128, SKT, PS), dtype=np.float32)
        w2bd = np.zeros((128, SKT), dtype=np.float32)
        biases = np.zeros((128, NB), dtype=np.float32)
        biases[:, :MH] = b_shared.reshape(MH, 128).T
        b2row = np.zeros((1, BL), dtype=np.float32)
        xparts = []
        for p in range(NPOS):
            g = order[p * N_CORES + c]
            toks = perm[g * PS : (g + 1) * PS]
            tok_order_parts.append(toks)
            # partition-major x chunk: [128, KD, PS]
            xc = x_s[g * PS : (g + 1) * PS]                    # [PS, D]
            xparts.append(
                np.ascontiguousarray(xc.T)
                .reshape(KD, 128, PS)
                .transpose(1, 0, 2)
                .ravel()
            )
            r = routes_s[g * PS : (g + 1) * PS]                # [PS, K]
            el = chex[g]
            ecp, kt = pattern[p], kt3[p]
            slots = np.full(ecp, -1, dtype=np.int64)
            slots[: len(el)] = el

            b1blk = np.zeros(kt * 128, dtype=np.float32)
            w2blk = np.zeros(kt * 128, dtype=np.float32)
            for j, e in enumerate(slots):
                if e < 0:
                    continue
                w1all[:, col0[p] + j * F : col0[p] + (j + 1) * F] = W1[e]
                b1blk[j * F : (j + 1) * F] = b1[e]
                w2blk[j * F : (j + 1) * F] = W2[e]
            biases[:, MH + boff[p] : MH + boff[p] + kt] = (
                b1blk.reshape(kt, 128).T
            )
            w2bd[:, boff[p] : boff[p] + kt] = w2blk.reshape(kt, 128).T

            # routing mask per slot, expanded to the f' rows of each slot
            mrow = np.zeros((ecp, PS), dtype=np.float32)
            for kk in range(r.shape[1]):
                mrow += (slots[:, None] == r[None, :, kk]) / r.shape[1]
            mflat = np.zeros((kt * 128, PS), dtype=np.float32)
            for j in range(ecp):
                mflat[j * F : (j + 1) * F] = mrow[j]
            mexp[:, boff[p] : boff[p] + kt, :] = (
                mflat.reshape(kt, 128, PS).transpose(1, 0, 2)
            )
            b2row[0, p * PS : (p + 1) * PS] = b2[r].mean(axis=1)

        xT = np.concatenate(xparts).astype(np_dt)
        im = {
            "xT": xT,
            "wsh": wsh,
            "w1all": w1all.astype(np_dt),
            "mexp": np.ascontiguousarray(mexp.reshape(128, SKT * PS)).astype(
                np_dt
            ),
            "w2bd": w2bd.astype(np_dt),
        }
        if not zero_bias:
            im["biases"] = biases
        if not zero_b2:
            im["b2row"] = b2row
        in_maps.append(im)
    tok_order = np.concatenate(tok_order_parts)
    return key, in_maps, tok_order


def kernel(**inputs) -> np.ndarray:
    key, in_maps, tok_order = prepare(inputs)
    nc = get_nc(key)
    res = run_bass_kernel_spmd(nc, in_maps, list(range(N_CORES)))
    out_sorted = np.concatenate([res.results[c]["out"] for c in range(N_CORES)])
    out = np.empty(B, dtype=np.float32)
    out[tok_order] = out_sorted
    return out.reshape(B, 1)
